# revision 9
# baseline (speedup 1.0000x reference)
"""CPC loss kernel for Trainium2, data-parallel over 8 NeuronCores.

Math
----
Reference (per row x of shape [C], target t, y = x[t], C = 128):
  ce   = logsumexp(x) - y
  bdc  = -(sum_{j != t} log_sigmoid(y - x_j)) / (C-1)
  bec  = -(0.5 * sum_{j,k in rest} log_sigmoid(x_j - x_k + EPS)) / ((C-1)(C-2))

With log_sigmoid(d) = -softplus(-d), extending the rest-pair sums to the full
C x C square plus O(C) corrections (EPS = 1e-10 is invisible in fp32):

  P1 = sum_j sp(x_j - y),  P2 = sum_j sp(y - x_j)     # full C each
  SP = sum_{j,k in C x C} sp(x_j - x_k)               # diagonal included
  row_loss = (mx + ln(sum e^{x-mx}) - y)
           + (P1 - log2)/(C-1) + 0.5*(SP - P1 - P2 + log2)/((C-1)(C-2))

The ACT tables in this toolchain have no softplus, so sp comes from
Exp + Ln(bias=1) (one table set: natural_log_exp_and_others), and the pair
count is halved with sp(d) + sp(-d) = 2*ln(1+e^d) - d:

  SP = 2*LNS - LC,   LNS = sum_{j<k} ln(1+e^{d_jk}) + npad*ln2  (measured,
       with npad = 64 zero pad columns; diagonal C*ln2 = 2*npad*ln2 cancels)
  LC = sum_i x_i * (C-1-2i)          # linear, on VectorE

Kernel structure (per core, 2048 rows as 16 batches of 128):
  - TensorE: D[r, f] = x_r,j(f) - x_r,k(f) over the 8128 j<k pairs (+64 pad)
    via lhsT = X^T (bf16) times constant W[kappa, f] = delta(kappa, j(f)) -
    delta(kappa, k(f)), into [128, 2048] PSUM chunks.
  - ScalarE: Exp then Ln(bias=1, accum_out) in-place on each PSUM chunk;
    P1/P2 via the per-partition bias port (bias = -y / +y); CE sumexp.
  - VectorE: max, target gather (iota == t mask), LC, final combine.
Per-row losses are DMA'd out; the host sums across rows and cores.
"""

import functools

import numpy as np
import ml_dtypes

import concourse.bass as bass
import concourse.tile as tile
import concourse.hw_specs as hw_specs
from concourse import bacc, mybir
from concourse.bass_utils import run_bass_kernel_spmd

# The act-table chooser greedily picks the first set containing each
# function, so an exp/ln-alternating kernel loads exp_and_others and
# natural_log in alternation (~2.7us per load, ~96 loads here). Blank the
# single-function sets (keeping dict order, so act_func_set_id indices into
# act_info.json stay valid) so both exp and ln resolve to
# natural_log_exp_and_others and a single load suffices.
_orig_get_activation_tables = hw_specs.get_activation_tables


@functools.cache
def _patched_activation_tables(module_arch: str):
    d = dict(_orig_get_activation_tables(module_arch))
    for name in ("exp_and_others", "natural_log", "exp_and_friends"):
        if name in d:
            d[name] = set()
    return d


hw_specs.get_activation_tables = _patched_activation_tables
bacc.get_activation_tables = _patched_activation_tables

N, C = 16384, 128
NCORES = 8
ROWS = N // NCORES            # rows per core
P = 128                       # partitions / rows per batch
NB = ROWS // P                # batches per core
NPAIR = (C * (C - 1)) // 2    # 8128
NPAD = 64
NF = NPAIR + NPAD             # 8192 pair columns
CHUNK = 2048                  # free elems per ACT instruction (4 PSUM banks)
NCHUNK = NF // CHUNK          # 4 chunks per batch
MM_N = 512                    # moving free dim per matmul (1 PSUM bank)

F32 = mybir.dt.float32
BF16 = mybir.dt.bfloat16
AF = mybir.ActivationFunctionType
ALU = mybir.AluOpType

LOG2 = float(np.log(2.0))
C_P1 = 1.0 / (C - 1) - 0.5 / ((C - 1) * (C - 2))
C_P2 = -0.5 / ((C - 1) * (C - 2))
C_SP = 0.5 / ((C - 1) * (C - 2))
C_CONST = -LOG2 / (C - 1) + 0.5 * LOG2 / ((C - 1) * (C - 2))

_cache: dict = {}


def _build_program() -> bass.Bass:
    # Bacc (not raw Bass): its compile() runs generate_event_semaphores,
    # which splits multi-sem waits (the ACT ISA has a single wait slot).
    nc = bacc.Bacc("TRN2")

    x_d = nc.declare_dram_parameter("x", [ROWS, C], F32, isOutput=False)
    xt_d = nc.declare_dram_parameter("xt", [C, ROWS], BF16, isOutput=False)
    w_d = nc.declare_dram_parameter("w", [C, NF], BF16, isOutput=False)
    io_d = nc.declare_dram_parameter("io", [P, C], F32, isOutput=False)
    cf_d = nc.declare_dram_parameter("cf", [P, C], F32, isOutput=False)
    tf_d = nc.declare_dram_parameter("tf", [ROWS], F32, isOutput=False)
    out_d = nc.declare_dram_parameter("out", [ROWS], F32, isOutput=True)

    with tile.TileContext(nc) as tc:
        with (
            tc.tile_pool(name="const", bufs=1) as const_pool,
            tc.tile_pool(name="work", bufs=3) as work,
            tc.tile_pool(name="acc", bufs=1) as acc_pool,
            tc.tile_pool(name="psum", bufs=2, space="PSUM") as psum_pool,
        ):
            w_sb = const_pool.tile([C, NF], BF16)
            nc.sync.dma_start(out=w_sb, in_=w_d[:])
            x_sb = const_pool.tile([P, NB, C], F32)
            nc.sync.dma_start(out=x_sb, in_=x_d.rearrange("(b p) c -> p b c", p=P))
            xt_sb = const_pool.tile([C, ROWS], BF16)
            nc.sync.dma_start(out=xt_sb, in_=xt_d[:])
            io_sb = const_pool.tile([P, C], F32)
            nc.sync.dma_start(out=io_sb, in_=io_d[:])
            cf_sb = const_pool.tile([P, C], F32)
            nc.sync.dma_start(out=cf_sb, in_=cf_d[:])
            t_sb = const_pool.tile([P, NB], F32)
            nc.sync.dma_start(out=t_sb, in_=tf_d.rearrange("(b p) -> p b", p=P))

            LNS = acc_pool.tile([P, NB], F32)
            LC = acc_pool.tile([P, NB], F32)
            P1 = acc_pool.tile([P, NB], F32)
            P2 = acc_pool.tile([P, NB], F32)
            MX = acc_pool.tile([P, NB], F32)
            NMX = acc_pool.tile([P, NB], F32)
            SE = acc_pool.tile([P, NB], F32)
            Y = acc_pool.tile([P, NB], F32)
            NY = acc_pool.tile([P, NB], F32)

            for b in range(NB):
                xb = x_sb[:, b, :]
                yb = Y[:, b : b + 1]
                nyb = NY[:, b : b + 1]

                nc.vector.tensor_reduce(
                    MX[:, b : b + 1], xb, axis=mybir.AxisListType.X, op=ALU.max
                )
                nc.vector.tensor_scalar_mul(NMX[:, b : b + 1], MX[:, b : b + 1], -1.0)

                # y = x[r, t_r] via (iota == t) mask then masked row-sum
                # (tensor_tensor_reduce is a custom DVE op that dies at
                # runtime here, so use plain mul + reduce)
                mask = work.tile([P, C], F32, tag="mask")
                nc.vector.tensor_scalar(
                    mask, io_sb, t_sb[:, b : b + 1], None, op0=ALU.is_equal
                )
                nc.vector.tensor_mul(mask, mask, xb)
                nc.vector.tensor_reduce(
                    yb, mask, axis=mybir.AxisListType.X, op=ALU.add
                )
                nc.vector.tensor_scalar_mul(nyb, yb, -1.0)

                # LC = sum_i x_i * (C-1-2i)
                prod = work.tile([P, C], F32, tag="prod")
                nc.vector.tensor_mul(prod, xb, cf_sb)
                nc.vector.tensor_reduce(
                    LC[:, b : b + 1], prod, axis=mybir.AxisListType.X, op=ALU.add
                )

                # P1 = sum_j sp(x_j - y) = sum_j ln(1 + e^{x_j - y})
                scr1 = work.tile([P, C], F32, tag="scr1")
                nc.scalar.activation(scr1, xb, AF.Exp, bias=nyb, scale=1.0)
                scr1b = work.tile([P, C], F32, tag="scr1b")
                nc.scalar.activation(
                    scr1b, scr1, AF.Ln, bias=1.0, scale=1.0,
                    accum_out=P1[:, b : b + 1],
                )
                # P2 = sum_j sp(y - x_j)
                scr2 = work.tile([P, C], F32, tag="scr2")
                nc.scalar.activation(scr2, xb, AF.Exp, bias=yb, scale=-1.0)
                scr2b = work.tile([P, C], F32, tag="scr2b")
                nc.scalar.activation(
                    scr2b, scr2, AF.Ln, bias=1.0, scale=1.0,
                    accum_out=P2[:, b : b + 1],
                )
                # CE sumexp
                scr3 = work.tile([P, C], F32, tag="scr3")
                nc.scalar.activation(
                    scr3, xb, AF.Exp, bias=NMX[:, b : b + 1], scale=1.0,
                    accum_out=SE[:, b : b + 1],
                )

                # LNS over the 8192 pair columns
                lnacc = work.tile([P, NCHUNK], F32, tag="lnacc")
                lhsT = xt_sb[:, b * P : (b + 1) * P]
                for ch in range(NCHUNK):
                    pt = psum_pool.tile([P, CHUNK], F32, tag="pair")
                    for m in range(CHUNK // MM_N):
                        f0 = ch * CHUNK + m * MM_N
                        nc.tensor.matmul(
                            pt[:, m * MM_N : (m + 1) * MM_N],
                            lhsT,
                            w_sb[:, f0 : f0 + MM_N],
                        )
                    nc.scalar.activation(pt, pt, AF.Exp, bias=0.0, scale=1.0)
                    nc.scalar.activation(
                        pt, pt, AF.Ln, bias=1.0, scale=1.0,
                        accum_out=lnacc[:, ch : ch + 1],
                    )
                nc.vector.tensor_reduce(
                    LNS[:, b : b + 1], lnacc, axis=mybir.AxisListType.X, op=ALU.add
                )

            LSE = acc_pool.tile([P, NB], F32)
            nc.scalar.activation(LSE, SE, AF.Ln)

            # row_loss = MX + LSE - Y + C_P1*P1 + C_P2*P2
            #          + (2*C_SP)*LNS - C_SP*LC + C_CONST
            L = acc_pool.tile([P, NB], F32)
            T1 = acc_pool.tile([P, NB], F32)
            nc.vector.tensor_add(L, MX, LSE)
            nc.vector.tensor_sub(L, L, Y)
            nc.vector.tensor_scalar_mul(T1, P1, C_P1)
            nc.vector.tensor_add(L, L, T1)
            nc.vector.tensor_scalar_mul(T1, P2, C_P2)
            nc.vector.tensor_add(L, L, T1)
            nc.vector.tensor_scalar_mul(T1, LNS, 2.0 * C_SP)
            nc.vector.tensor_add(L, L, T1)
            nc.vector.tensor_scalar_mul(T1, LC, -C_SP)
            nc.vector.tensor_add(L, L, T1)
            nc.vector.tensor_scalar_add(L, L, C_CONST)

            nc.sync.dma_start(out=out_d.rearrange("(b p) -> p b", p=P), in_=L)

    nc.compile()
    return nc


def _host_constants():
    if "w" not in _cache:
        ju, ku = np.triu_indices(C, 1)
        w = np.zeros((C, NF), np.float32)
        f = np.arange(NPAIR)
        w[ju, f] = 1.0
        w[ku, f] = -1.0
        _cache["w"] = w.astype(ml_dtypes.bfloat16)
        _cache["io"] = np.broadcast_to(
            np.arange(C, dtype=np.float32), (P, C)
        ).copy()
        _cache["cf"] = np.broadcast_to(
            (C - 1 - 2 * np.arange(C)).astype(np.float32), (P, C)
        ).copy()
    return _cache["w"], _cache["io"], _cache["cf"]


def kernel(inputs: np.ndarray, targets: np.ndarray) -> np.ndarray:
    x = np.ascontiguousarray(np.asarray(inputs, dtype=np.float32))
    t = np.asarray(targets)
    assert x.shape == (N, C) and t.shape == (N,)

    if "nc" not in _cache:
        _cache["nc"] = _build_program()
    nc = _cache["nc"]
    w, io, cf = _host_constants()

    xt = np.ascontiguousarray(x.T).astype(ml_dtypes.bfloat16)
    tf = t.astype(np.float32)

    in_maps = []
    for c in range(NCORES):
        r0, r1 = c * ROWS, (c + 1) * ROWS
        in_maps.append(
            {
                "x": np.ascontiguousarray(x[r0:r1]),
                "xt": np.ascontiguousarray(xt[:, r0:r1]),
                "w": w,
                "io": io,
                "cf": cf,
                "tf": np.ascontiguousarray(tf[r0:r1]),
            }
        )

    res = run_bass_kernel_spmd(nc, in_maps, list(range(NCORES)))
    total = 0.0
    for c in range(NCORES):
        total += np.sum(res.results[c]["out"].astype(np.float64))
    return np.float32(total / N)


# revision 11
# speedup vs baseline: 29.5310x; 29.5310x over previous
"""CPC loss kernel for Trainium2, data-parallel over 8 NeuronCores.

Math
----
Reference (per row x of shape [C], target t, y = x[t], C = 128):
  ce   = logsumexp(x) - y
  bdc  = -(sum_{j != t} log_sigmoid(y - x_j)) / (C-1)
  bec  = -(0.5 * sum_{j,k in rest} log_sigmoid(x_j - x_k + EPS)) / ((C-1)(C-2))

With log_sigmoid(d) = -softplus(-d), extending the rest-pair sums to the full
C x C square plus O(C) corrections (EPS = 1e-10 is invisible in fp32):

  P1 = sum_j sp(x_j - y),  P2 = sum_j sp(y - x_j)     # full C each
  SP = sum_{j,k in C x C} sp(x_j - x_k)               # diagonal included
  row_loss = (mx + ln(sum e^{x-mx}) - y)
           + (P1 - log2)/(C-1) + 0.5*(SP - P1 - P2 + log2)/((C-1)(C-2))

The ACT tables in this toolchain have no softplus, so sp comes from
Exp + Ln(bias=1) (one table set: natural_log_exp_and_others), and the pair
count is halved with sp(d) + sp(-d) = 2*ln(1+e^d) - d:

  SP = 2*LNS - LC,   LNS = sum_{j<k} ln(1+e^{d_jk}) + npad*ln2  (measured,
       with npad = 64 zero pad columns; diagonal C*ln2 = 2*npad*ln2 cancels)
  LC = sum_i x_i * (C-1-2i)          # linear, on VectorE

Kernel structure (per core, 2048 rows as 16 batches of 128):
  - TensorE: D[r, f] = x_r,j(f) - x_r,k(f) over the 8128 j<k pairs (+64 pad)
    via lhsT = X^T (bf16) times constant W[kappa, f] = delta(kappa, j(f)) -
    delta(kappa, k(f)), into [128, 2048] PSUM chunks.
  - ScalarE: Exp then Ln(bias=1, accum_out) in-place on each PSUM chunk;
    P1/P2 via the per-partition bias port (bias = -y / +y); CE sumexp.
  - VectorE: max, target gather (iota == t mask), LC, final combine.
Per-row losses are DMA'd out; the host sums across rows and cores.
"""

import functools

import numpy as np
import ml_dtypes

import concourse.bass as bass
import concourse.tile as tile
import concourse.hw_specs as hw_specs
from concourse import bacc, mybir
from concourse.bass_utils import run_bass_kernel_spmd

# The act-table chooser greedily picks the first set containing each
# function, so an exp/ln-alternating kernel loads exp_and_others and
# natural_log in alternation (~2.7us per load, ~96 loads here). Blank the
# single-function sets (keeping dict order, so act_func_set_id indices into
# act_info.json stay valid) so both exp and ln resolve to
# natural_log_exp_and_others and a single load suffices.
_orig_get_activation_tables = hw_specs.get_activation_tables


@functools.cache
def _patched_activation_tables(module_arch: str):
    d = dict(_orig_get_activation_tables(module_arch))
    for name in ("exp_and_others", "natural_log", "exp_and_friends"):
        if name in d:
            d[name] = set()
    return d


hw_specs.get_activation_tables = _patched_activation_tables
bacc.get_activation_tables = _patched_activation_tables

N, C = 16384, 128
NCORES = 8
ROWS = N // NCORES            # rows per core
P = 128                       # partitions / rows per batch
NB = ROWS // P                # batches per core
NPAIR = (C * (C - 1)) // 2    # 8128
NPAD = 64
NF = NPAIR + NPAD             # 8192 pair columns
CHUNK = 2048                  # free elems per ACT instruction (4 PSUM banks)
NCHUNK = NF // CHUNK          # 4 chunks per batch
MM_N = 512                    # moving free dim per matmul (1 PSUM bank)

F32 = mybir.dt.float32
BF16 = mybir.dt.bfloat16
AF = mybir.ActivationFunctionType
ALU = mybir.AluOpType

LOG2 = float(np.log(2.0))
C_P1 = 1.0 / (C - 1) - 0.5 / ((C - 1) * (C - 2))
C_P2 = -0.5 / ((C - 1) * (C - 2))
C_SP = 0.5 / ((C - 1) * (C - 2))
C_CONST = -LOG2 / (C - 1) + 0.5 * LOG2 / ((C - 1) * (C - 2))

_cache: dict = {}


def _build_program(repeat: int = 1) -> bass.Bass:
    # Bacc (not raw Bass): its compile() runs generate_event_semaphores,
    # which splits multi-sem waits (the ACT ISA has a single wait slot).
    nc = bacc.Bacc("TRN2")

    x_d = nc.declare_dram_parameter("x", [ROWS, C], F32, isOutput=False)
    xt_d = nc.declare_dram_parameter("xt", [C, ROWS], BF16, isOutput=False)
    w_d = nc.declare_dram_parameter("w", [C, NF], BF16, isOutput=False)
    io_d = nc.declare_dram_parameter("io", [P, C], F32, isOutput=False)
    cf_d = nc.declare_dram_parameter("cf", [P, C], F32, isOutput=False)
    tf_d = nc.declare_dram_parameter("tf", [ROWS], F32, isOutput=False)
    out_d = nc.declare_dram_parameter("out", [ROWS], F32, isOutput=True)

    with tile.TileContext(nc) as tc:
        with (
            tc.tile_pool(name="const", bufs=1) as const_pool,
            tc.tile_pool(name="work", bufs=3) as work,
            tc.tile_pool(name="acc", bufs=1) as acc_pool,
            tc.tile_pool(name="psum", bufs=2, space="PSUM") as psum_pool,
        ):
            w_sb = const_pool.tile([C, NF], BF16)
            nc.sync.dma_start(out=w_sb, in_=w_d[:])
            x_sb = const_pool.tile([P, NB, C], F32)
            nc.sync.dma_start(out=x_sb, in_=x_d.rearrange("(b p) c -> p b c", p=P))
            xt_sb = const_pool.tile([C, ROWS], BF16)
            nc.sync.dma_start(out=xt_sb, in_=xt_d[:])
            io_sb = const_pool.tile([P, C], F32)
            nc.sync.dma_start(out=io_sb, in_=io_d[:])
            cf_sb = const_pool.tile([P, C], F32)
            nc.sync.dma_start(out=cf_sb, in_=cf_d[:])
            t_sb = const_pool.tile([P, NB], F32)
            nc.sync.dma_start(out=t_sb, in_=tf_d.rearrange("(b p) -> p b", p=P))

            LNS = acc_pool.tile([P, NB], F32)
            LC = acc_pool.tile([P, NB], F32)
            P1 = acc_pool.tile([P, NB], F32)
            P2 = acc_pool.tile([P, NB], F32)
            MX = acc_pool.tile([P, NB], F32)
            NMX = acc_pool.tile([P, NB], F32)
            SE = acc_pool.tile([P, NB], F32)
            Y = acc_pool.tile([P, NB], F32)
            NY = acc_pool.tile([P, NB], F32)

            for _rep in range(repeat):
              for b in range(NB):
                xb = x_sb[:, b, :]
                yb = Y[:, b : b + 1]
                nyb = NY[:, b : b + 1]

                nc.vector.tensor_reduce(
                    MX[:, b : b + 1], xb, axis=mybir.AxisListType.X, op=ALU.max
                )
                nc.vector.tensor_scalar_mul(NMX[:, b : b + 1], MX[:, b : b + 1], -1.0)

                # y = x[r, t_r] via (iota == t) mask then masked row-sum
                # (tensor_tensor_reduce is a custom DVE op that dies at
                # runtime here, so use plain mul + reduce)
                mask = work.tile([P, C], F32, tag="mask")
                nc.vector.tensor_scalar(
                    mask, io_sb, t_sb[:, b : b + 1], None, op0=ALU.is_equal
                )
                nc.vector.tensor_mul(mask, mask, xb)
                nc.vector.tensor_reduce(
                    yb, mask, axis=mybir.AxisListType.X, op=ALU.add
                )
                nc.vector.tensor_scalar_mul(nyb, yb, -1.0)

                # LC = sum_i x_i * (C-1-2i)
                prod = work.tile([P, C], F32, tag="prod")
                nc.vector.tensor_mul(prod, xb, cf_sb)
                nc.vector.tensor_reduce(
                    LC[:, b : b + 1], prod, axis=mybir.AxisListType.X, op=ALU.add
                )

                # P1 = sum_j sp(x_j - y) = sum_j ln(1 + e^{x_j - y})
                scr1 = work.tile([P, C], F32, tag="scr1")
                nc.scalar.activation(scr1, xb, AF.Exp, bias=nyb, scale=1.0)
                scr1b = work.tile([P, C], F32, tag="scr1b")
                nc.scalar.activation(
                    scr1b, scr1, AF.Ln, bias=1.0, scale=1.0,
                    accum_out=P1[:, b : b + 1],
                )
                # P2 = sum_j sp(y - x_j)
                scr2 = work.tile([P, C], F32, tag="scr2")
                nc.scalar.activation(scr2, xb, AF.Exp, bias=yb, scale=-1.0)
                scr2b = work.tile([P, C], F32, tag="scr2b")
                nc.scalar.activation(
                    scr2b, scr2, AF.Ln, bias=1.0, scale=1.0,
                    accum_out=P2[:, b : b + 1],
                )
                # CE sumexp
                scr3 = work.tile([P, C], F32, tag="scr3")
                nc.scalar.activation(
                    scr3, xb, AF.Exp, bias=NMX[:, b : b + 1], scale=1.0,
                    accum_out=SE[:, b : b + 1],
                )

                # LNS over the 8192 pair columns
                lnacc = work.tile([P, NCHUNK], F32, tag="lnacc")
                lhsT = xt_sb[:, b * P : (b + 1) * P]
                for ch in range(NCHUNK):
                    pt = psum_pool.tile([P, CHUNK], F32, tag="pair")
                    for m in range(CHUNK // MM_N):
                        f0 = ch * CHUNK + m * MM_N
                        nc.tensor.matmul(
                            pt[:, m * MM_N : (m + 1) * MM_N],
                            lhsT,
                            w_sb[:, f0 : f0 + MM_N],
                        )
                    nc.scalar.activation(pt, pt, AF.Exp, bias=0.0, scale=1.0)
                    nc.scalar.activation(
                        pt, pt, AF.Ln, bias=1.0, scale=1.0,
                        accum_out=lnacc[:, ch : ch + 1],
                    )
                nc.vector.tensor_reduce(
                    LNS[:, b : b + 1], lnacc, axis=mybir.AxisListType.X, op=ALU.add
                )

            LSE = acc_pool.tile([P, NB], F32)
            nc.scalar.activation(LSE, SE, AF.Ln)

            # row_loss = MX + LSE - Y + C_P1*P1 + C_P2*P2
            #          + (2*C_SP)*LNS - C_SP*LC + C_CONST
            L = acc_pool.tile([P, NB], F32)
            T1 = acc_pool.tile([P, NB], F32)
            nc.vector.tensor_add(L, MX, LSE)
            nc.vector.tensor_sub(L, L, Y)
            nc.vector.tensor_scalar_mul(T1, P1, C_P1)
            nc.vector.tensor_add(L, L, T1)
            nc.vector.tensor_scalar_mul(T1, P2, C_P2)
            nc.vector.tensor_add(L, L, T1)
            nc.vector.tensor_scalar_mul(T1, LNS, 2.0 * C_SP)
            nc.vector.tensor_add(L, L, T1)
            nc.vector.tensor_scalar_mul(T1, LC, -C_SP)
            nc.vector.tensor_add(L, L, T1)
            nc.vector.tensor_scalar_add(L, L, C_CONST)

            nc.sync.dma_start(out=out_d.rearrange("(b p) -> p b", p=P), in_=L)

    nc.compile()
    return nc


def _host_constants():
    if "w" not in _cache:
        ju, ku = np.triu_indices(C, 1)
        w = np.zeros((C, NF), np.float32)
        f = np.arange(NPAIR)
        w[ju, f] = 1.0
        w[ku, f] = -1.0
        _cache["w"] = w.astype(ml_dtypes.bfloat16)
        _cache["io"] = np.broadcast_to(
            np.arange(C, dtype=np.float32), (P, C)
        ).copy()
        _cache["cf"] = np.broadcast_to(
            (C - 1 - 2 * np.arange(C)).astype(np.float32), (P, C)
        ).copy()
    return _cache["w"], _cache["io"], _cache["cf"]


def kernel(inputs: np.ndarray, targets: np.ndarray) -> np.ndarray:
    x = np.ascontiguousarray(np.asarray(inputs, dtype=np.float32))
    t = np.asarray(targets)
    assert x.shape == (N, C) and t.shape == (N,)

    if "nc" not in _cache:
        _cache["nc"] = _build_program()
    nc = _cache["nc"]
    w, io, cf = _host_constants()

    xt = np.ascontiguousarray(x.T).astype(ml_dtypes.bfloat16)
    tf = t.astype(np.float32)

    in_maps = []
    for c in range(NCORES):
        r0, r1 = c * ROWS, (c + 1) * ROWS
        in_maps.append(
            {
                "x": np.ascontiguousarray(x[r0:r1]),
                "xt": np.ascontiguousarray(xt[:, r0:r1]),
                "w": w,
                "io": io,
                "cf": cf,
                "tf": np.ascontiguousarray(tf[r0:r1]),
            }
        )

    res = run_bass_kernel_spmd(nc, in_maps, list(range(NCORES)))
    total = 0.0
    for c in range(NCORES):
        total += np.sum(res.results[c]["out"].astype(np.float64))
    return np.float32(total / N)
